# revision 28
# baseline (speedup 1.0000x reference)
"""Trainium2 Bass kernel: 16-head attention (S=1024, hidden=1024) + output
linear, data-parallel over the batch dimension (8 batch elements -> 8 cores).

Contract: kernel(**inputs) takes the FULL unsharded inputs of
nn_Attention_83915071029891 and returns the FULL (8, 1024, 1024) f32 output.

Per-core algorithm (transposed-scores layout; TensorE contracts over the
partition dim, so scores are built k-major and never transposed):
  per head-pair (head A on partitions 0-63, head B on 64-127):
    scoresT[ktile] = kT-stationary.T @ qT        (row-packed A||B matmuls)
    attnT = exp(scoresT / sqrt(1024))            (ScalarE, reads PSUM)
    outTaug = [v | 1]-stationary.T @ attnT       (PSUM-accumulated over k)
      rows 0-63: unnormalized out^T; row 64: softmax denominator
    rows scaled by reciprocal(denominator)       (DVE + broadcast DMA)
  y[qtile] = outT-stationary.T @ fc_wT + fc_b    (PSUM-accumulated over i)
"""

import sys

for _p in ("/opt/trn_rl_repo", "/root/.axon_site/_ro/trn_rl_repo"):
    if _p not in sys.path:
        sys.path.append(_p)

from contextlib import ExitStack

import numpy as np

import bass_rust
import concourse.bass as bass
import concourse.mybir as mybir
import concourse.tile as tile
from concourse import library_config
from concourse.vector_clock import ScopedClock

F32 = mybir.dt.float32
AF = mybir.ActivationFunctionType

N_CORES = 8
_MAX_CTRL_WAITS = 1
ROWPACK_DEP = True


def _patched_drain_and_barrier(self, tick_clock, wait_clock):
    """Tile's kernel-tail Drain aggregates one sem wait per outstanding proc,
    but walrus CoreV3 codegen only has one sync-wait slot on CTRL ops -- split
    the waits across a chain of SP drain instructions."""
    nc = self.nc
    drain_inst = nc.sync.drain()
    wait_clock.add_sem_waits(
        drain_inst.ins, ScopedClock({None: tick_clock.global_clock})
    )
    si = drain_inst.ins.sync_info
    if si is not None and si.on_wait and len(si.on_wait) > _MAX_CTRL_WAITS:
        waits = list(si.on_wait)
        drain_inst.ins.sync_info = bass_rust.SyncInfo(
            on_wait=waits[:_MAX_CTRL_WAITS], on_update=list(si.on_update or [])
        )
        for i in range(_MAX_CTRL_WAITS, len(waits), _MAX_CTRL_WAITS):
            extra = nc.sync.drain()
            extra.ins.sync_info = bass_rust.SyncInfo(
                on_wait=waits[i : i + _MAX_CTRL_WAITS], on_update=[]
            )

    nc.all_engine_barrier()
    assert self.sems is not None
    popped = nc._tile_sem_poison_stack.pop()
    assert popped is self._sem_poison
    nc.clear_and_free_semaphores(list(self.sems.allocated().values()))
    nc.all_engine_barrier()


tile.TileContext._drain_and_barrier = _patched_drain_and_barrier


def _split_excess_waits(nc, max_waits=_MAX_CTRL_WAITS):
    """walrus CoreV3 setupSyncWait only has one sync-wait slot per
    instruction; hoist excess sem waits onto same-engine NoOp carriers
    inserted immediately before the over-limit instruction."""
    ctr = [0]

    def carrier(engine, waits):
        ctr[0] += 1
        nop = mybir.InstNoOp(name=f"I-waitc-{ctr[0]}", ins=[], outs=[])
        nop.engine = engine
        nop.sync_info = bass_rust.SyncInfo(on_wait=waits, on_update=[])
        return nop

    for fn in nc.m.functions:
        for blk in fn.blocks:
            il = blk.instructions
            newl = []
            changed = False
            for inst in il:
                si = inst.sync_info
                nw = len(si.on_wait) if si and si.on_wait else 0
                if nw > max_waits:
                    waits = list(si.on_wait)
                    for i in range(max_waits, len(waits), max_waits):
                        newl.append(carrier(inst.engine, waits[i : i + max_waits]))
                    inst.sync_info = bass_rust.SyncInfo(
                        on_wait=waits[:max_waits], on_update=list(si.on_update or [])
                    )
                    changed = True
                newl.append(inst)
            if changed:
                il.clear()
                il.extend(newl)
                assert len(blk.instructions) == len(newl), (
                    "block instruction list is not a live reference"
                )


def _splits(total, width):
    return [(s, min(width, total - s)) for s in range(0, total, width)]


def build_kernel(S=1024, HEADS=16, mm_dtype="f32r", split_waits=True):
    """Trace the per-core Bass program. DRAM io: qT,kT,vaug,fc_wT,fc_b -> y."""
    HD = 64
    H = HEADS * HD
    KT = S // 128
    PAIRS = HEADS // 2
    ITILES = H // 128
    QTILES = S // 128
    VW = HD + 1
    SCALE = 1.0 / float(H) ** 0.5
    NSP = _splits(S, 512)
    OSP = _splits(H, 512)

    nc = bass.Bass(trn_type="TRN2")

    # fp32 matmuls run as 2 half-speed passes (4 cyc/row). fp32r streams at
    # 1 cyc/row but does not register as PE activity for the HAM clock gate,
    # pinning the array at 1.2 GHz. bf16 runs the normal warm path.
    MMDT = {"f32r": mybir.dt.float32r, "bf16": mybir.dt.bfloat16, "f32": F32}[mm_dtype]
    # reciprocal-broadcast matmul operands: f32r keeps the full fp32 bytes
    # (bf16 would cost ~0.4% on every normalized output)
    RDT = F32 if mm_dtype == "f32" else mybir.dt.float32r

    qT = nc.dram_tensor("qT", [H, S], MMDT, kind="ExternalInput").ap()
    kT = nc.dram_tensor("kT", [H, S], MMDT, kind="ExternalInput").ap()
    vaug = nc.dram_tensor("vaug", [HEADS, 128, KT * VW], MMDT, kind="ExternalInput").ap()
    fc_wT = nc.dram_tensor("fc_wT", [H, H], MMDT, kind="ExternalInput").ap()
    fc_b = nc.dram_tensor("fc_b", [1, H], F32, kind="ExternalInput").ap()
    ones64 = nc.dram_tensor("ones64", [1, 64], RDT, kind="ExternalInput").ap()
    y = nc.dram_tensor("y", [S, H], F32, kind="ExternalOutput").ap()

    with tile.TileContext(nc) as tc:
        with ExitStack() as ctx:
            big = ctx.enter_context(tc.tile_pool(name="big", bufs=1))
            qk = ctx.enter_context(tc.tile_pool(name="qk", bufs=2))
            vp = ctx.enter_context(tc.tile_pool(name="vp", bufs=2))
            at = ctx.enter_context(tc.tile_pool(name="at", bufs=6))
            tl = ctx.enter_context(tc.tile_pool(name="tl", bufs=3))
            yp = ctx.enter_context(tc.tile_pool(name="yp", bufs=2))
            # PSUM budget (8 banks): "ps" 2 x (128,S)=4 banks, "po" oA+oB=4
            ps = ctx.enter_context(tc.tile_pool(name="ps", bufs=3, space="PSUM"))
            po = ctx.enter_context(tc.tile_pool(name="po", bufs=1, space="PSUM"))

            outT_sb = big.tile([128, ITILES * S], MMDT, tag="outT")
            ones1 = big.tile([1, 64], RDT, tag="ones1")
            nc.sync.dma_start(out=ones1[:, :], in_=ones64[:, :])
            # fc weights ride the (otherwise idle) GPSIMD SWDGE ring so they
            # never queue ahead of the SP-ring attention input loads.
            fcw_sb = big.tile([128, ITILES * H], MMDT, tag="fcw")
            fcb_sb = big.tile([128, H], F32, tag="fcb")
            for i in range(ITILES):
                nc.gpsimd.dma_start(
                    out=fcw_sb[:, H * i : H * (i + 1)],
                    in_=fc_wT[128 * i : 128 * (i + 1), :],
                )
            nc.gpsimd.dma_start(
                out=fcb_sb[:, :], in_=fc_b.unsqueeze(1).broadcast_to((1, 128, H))
            )

            def load_pair(p):
                psl = slice(128 * p, 128 * (p + 1))
                qTp = qk.tile([128, S], MMDT, tag="qT")
                kTp = qk.tile([128, S], MMDT, tag="kT")
                if p == 0:
                    # small first chunks so the first QK matmul starts early
                    c0 = min(512, S)
                    nc.sync.dma_start(out=kTp[:, 0:128], in_=kT[psl, 0:128])
                    nc.sync.dma_start(out=qTp[:, 0:c0], in_=qT[psl, 0:c0])
                    nc.sync.dma_start(out=kTp[:, 128:S], in_=kT[psl, 128:S])
                    if c0 < S:
                        nc.sync.dma_start(out=qTp[:, c0:S], in_=qT[psl, c0:S])
                else:
                    nc.sync.dma_start(out=qTp[:, :], in_=qT[psl, :])
                    nc.sync.dma_start(out=kTp[:, :], in_=kT[psl, :])
                vA = vp.tile([128, KT * VW], MMDT, tag="vA")
                nc.sync.dma_start(out=vA[:, :], in_=vaug[2 * p])
                vB = vp.tile([128, KT * VW], MMDT, tag="vB")
                nc.sync.dma_start(out=vB[:, :], in_=vaug[2 * p + 1])
                return qTp, kTp, vA, vB

            nxt = load_pair(0)
            tails = []

            def emit_norm(entries):
                for stage, rec, hX, p_, n0, nw in entries:
                    # R = ones(64)^T @ rec : per-q reciprocal broadcast to 64
                    # partitions via a K=1 matmul into a spare PSUM slot
                    Rp = ps.tile([64, nw], F32, tag="s")
                    for m0, mw in _splits(nw, 512):
                        nc.tensor.matmul(
                            Rp[:, m0 : m0 + mw], ones1[:, :],
                            rec[0:1, m0 : m0 + mw], start=True, stop=True,
                        )
                    osl = slice(S * p_ + n0, S * p_ + n0 + nw)
                    nsl_ = slice(n0, n0 + nw)
                    if hX % 2 == 0:
                        nc.vector.tensor_mul(
                            outT_sb[0:64, osl], stage[0:64, nsl_], Rp[:, :]
                        )
                    else:
                        stg = tl.tile([64, nw], MMDT, tag="stg")
                        nc.vector.tensor_mul(stg[:, :], stage[0:64, nsl_], Rp[:, :])
                        nc.sync.dma_start(out=outT_sb[64:128, osl], in_=stg[:, :])

            for p in range(PAIRS):
                hA, hB = 2 * p, 2 * p + 1
                qTp, kTp, vA, vB = nxt
                if p + 1 < PAIRS:
                    nxt = load_pair(p + 1)
                stageA = tl.tile([65, S], F32, tag="stageA")
                stageB = tl.tile([65, S], F32, tag="stageB")
                # q-dim processed in 512-wide passes: each (ktile, pass)
                # emits one [scoresT_A | scoresT_B] PSUM tile so a single
                # FD=1024 ACT exp covers both heads, and the AV accumulators
                # only need one PSUM bank per head. Each pass is normalized
                # as soon as its AV accumulation finishes.
                for n0, nw in NSP:
                    nsl = slice(n0, n0 + nw)
                    po_A = po.tile([VW, nw], F32, tag="oA")
                    po_B = po.tile([VW, nw], F32, tag="oB")
                    for t in range(KT):
                        ksl = slice(128 * t, 128 * (t + 1))
                        sAB = ps.tile([128, 2 * nw], F32, tag="s")
                        mmA = nc.tensor.matmul(
                            sAB[:, 0:nw], kTp[0:64, ksl], qTp[0:64, nsl],
                            start=True, stop=True,
                        )
                        mmB = nc.tensor.matmul(
                            sAB[:, nw : 2 * nw], kTp[64:128, ksl], qTp[64:128, nsl],
                            start=True, stop=True,
                        )
                        # keep the K=64 row-group pair adjacent on PE so the
                        # two half-array matmuls run concurrently
                        if ROWPACK_DEP:
                            tile.add_dep_helper(
                                mmB.ins, mmA.ins, sync=False, reason="rowpack"
                            )
                        aAB = at.tile([128, 2 * nw], MMDT, tag="a")
                        nc.scalar.activation(aAB[:, :], sAB[:, :], AF.Exp, scale=SCALE)
                        vsl = slice(VW * t, VW * (t + 1))
                        nc.tensor.matmul(
                            po_A[:, :], vA[:, vsl], aAB[:, 0:nw],
                            start=(t == 0), stop=(t == KT - 1),
                        )
                        nc.tensor.matmul(
                            po_B[:, :], vB[:, vsl], aAB[:, nw : 2 * nw],
                            start=(t == 0), stop=(t == KT - 1),
                        )
                    for po_X, stage in ((po_A, stageA), (po_B, stageB)):
                        # evacuate the accumulator so the PSUM bank frees for
                        # the next q-pass; normalization happens per pair
                        nc.vector.tensor_copy(stage[:, nsl], po_X[:, :])

                for stage, hX in ((stageA, hA), (stageB, hB)):
                    # 1/denominator: reshape the row to (128, S/128) so the
                    # iterative RECIPROCAL walks few elements per lane
                    denP = tl.tile([128, S // 128], F32, tag="denP")
                    nc.sync.dma_start(
                        out=denP[:, :],
                        in_=stage[64:65, :].rearrange("p (a b) -> p a b", b=S // 128),
                    )
                    recP = tl.tile([128, S // 128], F32, tag="recP")
                    nc.vector.reciprocal(recP[:, :], denP[:, :])
                    rec = tl.tile([1, S], RDT, tag="rec")
                    # f32r is byte-identical to f32: bitcast instead of a
                    # cast-DMA so this rides the fast HWDGE ring
                    nc.sync.dma_start(
                        out=rec[:, :].rearrange("p (a b) -> p a b", b=S // 128),
                        in_=recP[:, :].bitcast(RDT) if RDT != F32 else recP[:, :],
                    )
                    tails.append((stage, rec, hX, p, 0, S))

                # normalization (broadcast matmul + multiply) deferred by one
                # pair: the PE queue is strictly in-order, so the K=1
                # broadcast matmul must never wait on a fresh rec chain
                if len(tails) > 2:
                    emit_norm(tails[:-2])
                    tails = tails[-2:]

            def fc_mms(py, q, i_range, last):
                for i in i_range:
                    lhsT = outT_sb[:, S * i + 128 * q : S * i + 128 * (q + 1)]
                    for o0, ow in OSP:
                        nc.tensor.matmul(
                            py[:, o0 : o0 + ow],
                            lhsT,
                            fcw_sb[:, H * i + o0 : H * i + o0 + ow],
                            start=(i == 0), stop=(i == last),
                        )

            # the final pairs' normalization matmuls wait on their rec
            # chains; fill that in-order PE bubble with the first qtile's
            # early fc accumulation (itiles 0..ITILES-3 are long since
            # normalized and independent of the pending norms)
            py0 = ps.tile([128, H], F32, tag="s")
            fc_mms(py0, 0, range(ITILES - 2), ITILES - 1)
            emit_norm(tails)
            fc_mms(py0, 0, range(ITILES - 2, ITILES), ITILES - 1)
            ysb0 = yp.tile([128, H], F32, tag="ysb")
            nc.vector.tensor_add(ysb0[:, :], py0[:, :], fcb_sb[:, :])
            nc.sync.dma_start(out=y[0:128, :], in_=ysb0[:, :])

            for q in range(1, QTILES):
                # per-half epilogue: half 0's bias-add + store overlap half
                # 1's matmuls, shrinking the exposed serial tail before the
                # kernel drain
                for o0, ow in OSP:
                    pyh = ps.tile([128, 512], F32, tag="s", name=f"pyh{q}_{o0}")
                    for i in range(ITILES):
                        nc.tensor.matmul(
                            pyh[:, 0:ow],
                            outT_sb[:, S * i + 128 * q : S * i + 128 * (q + 1)],
                            fcw_sb[:, H * i + o0 : H * i + o0 + ow],
                            start=(i == 0), stop=(i == ITILES - 1),
                        )
                    ysbh = yp.tile([128, 512], F32, tag="ysb", name=f"ysbh{q}_{o0}")
                    nc.vector.tensor_add(
                        ysbh[:, 0:ow], pyh[:, 0:ow], fcb_sb[:, o0 : o0 + ow]
                    )
                    nc.sync.dma_start(
                        out=y[128 * q : 128 * (q + 1), o0 : o0 + ow],
                        in_=ysbh[:, 0:ow],
                    )

    if split_waits:
        _split_excess_waits(nc)
    return nc


def prep_core_inputs(q_n, k_n, v_n, fc_wT, fc_b1, HEADS=16, mm_dtype="f32r"):
    """Host-side layout prep for one batch element."""
    import ml_dtypes

    cast = (lambda a: a.astype(ml_dtypes.bfloat16)) if mm_dtype == "bf16" else (lambda a: a)
    HD = 64
    S, H = q_n.shape
    KT = S // 128
    qT = np.ascontiguousarray(q_n.T)
    kT = np.ascontiguousarray(k_n.T)
    v4 = v_n.reshape(KT, 128, HEADS, HD)  # [t, p, h, c]
    vaug = np.empty((HEADS, 128, KT, HD + 1), dtype=np.float32)
    vaug[..., :HD] = v4.transpose(2, 1, 0, 3)
    vaug[..., HD] = 1.0
    return {
        "ones64": np.ones((1, 64), dtype=np.float32),
        "qT": cast(qT),
        "kT": cast(kT),
        "vaug": cast(np.ascontiguousarray(vaug.reshape(HEADS, 128, KT * (HD + 1)))),
        "fc_wT": cast(fc_wT),
        "fc_b": fc_b1,
    }


MM_DTYPE = "bf16"

_CACHED_NC = None


def _get_nc():
    global _CACHED_NC
    if _CACHED_NC is None:
        _CACHED_NC = build_kernel(mm_dtype=MM_DTYPE)
    return _CACHED_NC


def make_in_maps(key, value, query, fc_w, fc_b):
    key = np.asarray(key, dtype=np.float32)
    value = np.asarray(value, dtype=np.float32)
    query = np.asarray(query, dtype=np.float32)
    fc_w = np.asarray(fc_w, dtype=np.float32)
    fc_b = np.asarray(fc_b, dtype=np.float32)
    N, S, H = query.shape
    fc_wT = np.ascontiguousarray(fc_w.T)
    fc_b1 = np.ascontiguousarray(fc_b.reshape(1, H))
    return [
        prep_core_inputs(query[n], key[n], value[n], fc_wT, fc_b1, mm_dtype=MM_DTYPE)
        for n in range(N)
    ]


def run_on_device(in_maps):
    from concourse.bass_utils import run_bass_kernel_spmd

    nc = _get_nc()
    res = run_bass_kernel_spmd(nc, in_maps, list(range(N_CORES)))
    return np.stack([res.results[i]["y"] for i in range(N_CORES)], axis=0)


def kernel(key, value, query, fc_w, fc_b):
    """Full inputs in, full output out. Shards batch N=8 across 8 cores."""
    in_maps = make_in_maps(key, value, query, fc_w, fc_b)
    return run_on_device(in_maps)



# revision 29
# speedup vs baseline: 1.0219x; 1.0219x over previous
"""Trainium2 Bass kernel: 16-head attention (S=1024, hidden=1024) + output
linear, data-parallel over the batch dimension (8 batch elements -> 8 cores).

Contract: kernel(**inputs) takes the FULL unsharded inputs of
nn_Attention_83915071029891 and returns the FULL (8, 1024, 1024) f32 output.

Per-core algorithm (transposed-scores layout; TensorE contracts over the
partition dim, so scores are built k-major and never transposed):
  per head-pair (head A on partitions 0-63, head B on 64-127):
    scoresT[ktile] = kT-stationary.T @ qT        (row-packed A||B matmuls)
    attnT = exp(scoresT / sqrt(1024))            (ScalarE, reads PSUM)
    outTaug = [v | 1]-stationary.T @ attnT       (PSUM-accumulated over k)
      rows 0-63: unnormalized out^T; row 64: softmax denominator
    rows scaled by reciprocal(denominator)       (DVE + broadcast DMA)
  y[qtile] = outT-stationary.T @ fc_wT + fc_b    (PSUM-accumulated over i)
"""

import sys

for _p in ("/opt/trn_rl_repo", "/root/.axon_site/_ro/trn_rl_repo"):
    if _p not in sys.path:
        sys.path.append(_p)

from contextlib import ExitStack

import numpy as np

import bass_rust
import concourse.bass as bass
import concourse.mybir as mybir
import concourse.tile as tile
from concourse import library_config
from concourse.vector_clock import ScopedClock

F32 = mybir.dt.float32
AF = mybir.ActivationFunctionType

N_CORES = 8
_MAX_CTRL_WAITS = 1
ROWPACK_DEP = True


def _patched_drain_and_barrier(self, tick_clock, wait_clock):
    """Tile's kernel-tail Drain aggregates one sem wait per outstanding proc,
    but walrus CoreV3 codegen only has one sync-wait slot on CTRL ops -- split
    the waits across a chain of SP drain instructions."""
    nc = self.nc
    drain_inst = nc.sync.drain()
    wait_clock.add_sem_waits(
        drain_inst.ins, ScopedClock({None: tick_clock.global_clock})
    )
    si = drain_inst.ins.sync_info
    if si is not None and si.on_wait and len(si.on_wait) > _MAX_CTRL_WAITS:
        waits = list(si.on_wait)
        drain_inst.ins.sync_info = bass_rust.SyncInfo(
            on_wait=waits[:_MAX_CTRL_WAITS], on_update=list(si.on_update or [])
        )
        for i in range(_MAX_CTRL_WAITS, len(waits), _MAX_CTRL_WAITS):
            extra = nc.sync.drain()
            extra.ins.sync_info = bass_rust.SyncInfo(
                on_wait=waits[i : i + _MAX_CTRL_WAITS], on_update=[]
            )

    nc.all_engine_barrier()
    assert self.sems is not None
    popped = nc._tile_sem_poison_stack.pop()
    assert popped is self._sem_poison
    nc.clear_and_free_semaphores(list(self.sems.allocated().values()))
    nc.all_engine_barrier()


tile.TileContext._drain_and_barrier = _patched_drain_and_barrier


def _split_excess_waits(nc, max_waits=_MAX_CTRL_WAITS):
    """walrus CoreV3 setupSyncWait only has one sync-wait slot per
    instruction; hoist excess sem waits onto same-engine NoOp carriers
    inserted immediately before the over-limit instruction."""
    ctr = [0]

    def carrier(engine, waits):
        ctr[0] += 1
        nop = mybir.InstNoOp(name=f"I-waitc-{ctr[0]}", ins=[], outs=[])
        nop.engine = engine
        nop.sync_info = bass_rust.SyncInfo(on_wait=waits, on_update=[])
        return nop

    for fn in nc.m.functions:
        for blk in fn.blocks:
            il = blk.instructions
            newl = []
            changed = False
            for inst in il:
                si = inst.sync_info
                nw = len(si.on_wait) if si and si.on_wait else 0
                if nw > max_waits:
                    waits = list(si.on_wait)
                    for i in range(max_waits, len(waits), max_waits):
                        newl.append(carrier(inst.engine, waits[i : i + max_waits]))
                    inst.sync_info = bass_rust.SyncInfo(
                        on_wait=waits[:max_waits], on_update=list(si.on_update or [])
                    )
                    changed = True
                newl.append(inst)
            if changed:
                il.clear()
                il.extend(newl)
                assert len(blk.instructions) == len(newl), (
                    "block instruction list is not a live reference"
                )


def _splits(total, width):
    return [(s, min(width, total - s)) for s in range(0, total, width)]


def build_kernel(S=1024, HEADS=16, mm_dtype="f32r", split_waits=True):
    """Trace the per-core Bass program. DRAM io: qT,kT,vaug,fc_wT,fc_b -> y."""
    HD = 64
    H = HEADS * HD
    KT = S // 128
    PAIRS = HEADS // 2
    ITILES = H // 128
    QTILES = S // 128
    VW = HD + 1
    SCALE = 1.0 / float(H) ** 0.5
    NSP = _splits(S, 512)
    OSP = _splits(H, 512)

    nc = bass.Bass(trn_type="TRN2")

    # fp32 matmuls run as 2 half-speed passes (4 cyc/row). fp32r streams at
    # 1 cyc/row but does not register as PE activity for the HAM clock gate,
    # pinning the array at 1.2 GHz. bf16 runs the normal warm path.
    MMDT = {"f32r": mybir.dt.float32r, "bf16": mybir.dt.bfloat16, "f32": F32}[mm_dtype]
    # reciprocal-broadcast matmul operands: f32r keeps the full fp32 bytes
    # (bf16 would cost ~0.4% on every normalized output)
    RDT = F32 if mm_dtype == "f32" else mybir.dt.float32r

    qT = nc.dram_tensor("qT", [H, S], MMDT, kind="ExternalInput").ap()
    kT = nc.dram_tensor("kT", [H, S], MMDT, kind="ExternalInput").ap()
    vaug = nc.dram_tensor("vaug", [HEADS, 128, KT * VW], MMDT, kind="ExternalInput").ap()
    fc_wT = nc.dram_tensor("fc_wT", [H, H], MMDT, kind="ExternalInput").ap()
    fc_b = nc.dram_tensor("fc_b", [1, H], F32, kind="ExternalInput").ap()
    ones64 = nc.dram_tensor("ones64", [1, 64], RDT, kind="ExternalInput").ap()
    y = nc.dram_tensor("y", [S, H], F32, kind="ExternalOutput").ap()

    with tile.TileContext(nc) as tc:
        with ExitStack() as ctx:
            big = ctx.enter_context(tc.tile_pool(name="big", bufs=1))
            qk = ctx.enter_context(tc.tile_pool(name="qk", bufs=2))
            vp = ctx.enter_context(tc.tile_pool(name="vp", bufs=2))
            at = ctx.enter_context(tc.tile_pool(name="at", bufs=6))
            tl = ctx.enter_context(tc.tile_pool(name="tl", bufs=3))
            yp = ctx.enter_context(tc.tile_pool(name="yp", bufs=2))
            # PSUM budget (8 banks): "ps" 2 x (128,S)=4 banks, "po" oA+oB=4
            ps = ctx.enter_context(tc.tile_pool(name="ps", bufs=3, space="PSUM"))
            po = ctx.enter_context(tc.tile_pool(name="po", bufs=1, space="PSUM"))

            outT_sb = big.tile([128, ITILES * S], MMDT, tag="outT")
            ones1 = big.tile([1, 64], RDT, tag="ones1")
            nc.sync.dma_start(out=ones1[:, :], in_=ones64[:, :])
            # fc weights ride the (otherwise idle) GPSIMD SWDGE ring so they
            # never queue ahead of the SP-ring attention input loads.
            fcw_sb = big.tile([128, ITILES * H], MMDT, tag="fcw")
            fcb_sb = big.tile([128, H], F32, tag="fcb")
            for i in range(ITILES):
                nc.gpsimd.dma_start(
                    out=fcw_sb[:, H * i : H * (i + 1)],
                    in_=fc_wT[128 * i : 128 * (i + 1), :],
                )
            nc.gpsimd.dma_start(
                out=fcb_sb[:, :], in_=fc_b.unsqueeze(1).broadcast_to((1, 128, H))
            )

            def load_pair(p):
                psl = slice(128 * p, 128 * (p + 1))
                qTp = qk.tile([128, S], MMDT, tag="qT")
                kTp = qk.tile([128, S], MMDT, tag="kT")
                if p == 0:
                    # small first chunks so the first QK matmul starts early
                    c0 = min(512, S)
                    nc.sync.dma_start(out=kTp[:, 0:128], in_=kT[psl, 0:128])
                    nc.sync.dma_start(out=qTp[:, 0:c0], in_=qT[psl, 0:c0])
                    nc.sync.dma_start(out=kTp[:, 128:S], in_=kT[psl, 128:S])
                    if c0 < S:
                        nc.sync.dma_start(out=qTp[:, c0:S], in_=qT[psl, c0:S])
                else:
                    nc.sync.dma_start(out=qTp[:, :], in_=qT[psl, :])
                    nc.sync.dma_start(out=kTp[:, :], in_=kT[psl, :])
                vA = vp.tile([128, KT * VW], MMDT, tag="vA")
                nc.sync.dma_start(out=vA[:, :], in_=vaug[2 * p])
                vB = vp.tile([128, KT * VW], MMDT, tag="vB")
                nc.sync.dma_start(out=vB[:, :], in_=vaug[2 * p + 1])
                return qTp, kTp, vA, vB

            nxt = load_pair(0)
            tails = []

            def emit_norm(entries):
                for stage, rec, hX, p_, n0, nw in entries:
                    # R = ones(64)^T @ rec : per-q reciprocal broadcast to 64
                    # partitions via a K=1 matmul into a spare PSUM slot
                    Rp = ps.tile([64, nw], F32, tag="s")
                    for m0, mw in _splits(nw, 512):
                        nc.tensor.matmul(
                            Rp[:, m0 : m0 + mw], ones1[:, :],
                            rec[0:1, m0 : m0 + mw], start=True, stop=True,
                        )
                    osl = slice(S * p_ + n0, S * p_ + n0 + nw)
                    nsl_ = slice(n0, n0 + nw)
                    if hX % 2 == 0:
                        nc.vector.tensor_mul(
                            outT_sb[0:64, osl], stage[0:64, nsl_], Rp[:, :]
                        )
                    else:
                        stg = tl.tile([64, nw], MMDT, tag="stg")
                        nc.vector.tensor_mul(stg[:, :], stage[0:64, nsl_], Rp[:, :])
                        nc.sync.dma_start(out=outT_sb[64:128, osl], in_=stg[:, :])

            for p in range(PAIRS):
                hA, hB = 2 * p, 2 * p + 1
                qTp, kTp, vA, vB = nxt
                if p + 1 < PAIRS:
                    nxt = load_pair(p + 1)
                stageA = tl.tile([65, S], F32, tag="stageA")
                stageB = tl.tile([65, S], F32, tag="stageB")
                # q-dim processed in 512-wide passes: each (ktile, pass)
                # emits one [scoresT_A | scoresT_B] PSUM tile so a single
                # FD=1024 ACT exp covers both heads, and the AV accumulators
                # only need one PSUM bank per head. Each pass is normalized
                # as soon as its AV accumulation finishes.
                for n0, nw in NSP:
                    nsl = slice(n0, n0 + nw)
                    po_A = po.tile([VW, nw], F32, tag="oA")
                    po_B = po.tile([VW, nw], F32, tag="oB")
                    for t in range(KT):
                        ksl = slice(128 * t, 128 * (t + 1))
                        sAB = ps.tile([128, 2 * nw], F32, tag="s")
                        mmA = nc.tensor.matmul(
                            sAB[:, 0:nw], kTp[0:64, ksl], qTp[0:64, nsl],
                            start=True, stop=True,
                        )
                        mmB = nc.tensor.matmul(
                            sAB[:, nw : 2 * nw], kTp[64:128, ksl], qTp[64:128, nsl],
                            start=True, stop=True,
                        )
                        # keep the K=64 row-group pair adjacent on PE so the
                        # two half-array matmuls run concurrently
                        if ROWPACK_DEP:
                            tile.add_dep_helper(
                                mmB.ins, mmA.ins, sync=False, reason="rowpack"
                            )
                        aAB = at.tile([128, 2 * nw], MMDT, tag="a")
                        nc.scalar.activation(aAB[:, :], sAB[:, :], AF.Exp, scale=SCALE)
                        vsl = slice(VW * t, VW * (t + 1))
                        nc.tensor.matmul(
                            po_A[:, :], vA[:, vsl], aAB[:, 0:nw],
                            start=(t == 0), stop=(t == KT - 1),
                        )
                        nc.tensor.matmul(
                            po_B[:, :], vB[:, vsl], aAB[:, nw : 2 * nw],
                            start=(t == 0), stop=(t == KT - 1),
                        )
                    for po_X, stage in ((po_A, stageA), (po_B, stageB)):
                        # evacuate the accumulator so the PSUM bank frees for
                        # the next q-pass; normalization happens per pair
                        nc.vector.tensor_copy(stage[:, nsl], po_X[:, :])

                for stage, hX in ((stageA, hA), (stageB, hB)):
                    # 1/denominator: reshape the row to (128, S/128) so the
                    # iterative RECIPROCAL walks few elements per lane
                    denP = tl.tile([128, S // 128], F32, tag="denP")
                    nc.sync.dma_start(
                        out=denP[:, :],
                        in_=stage[64:65, :].rearrange("p (a b) -> p a b", b=S // 128),
                    )
                    recP = tl.tile([128, S // 128], F32, tag="recP")
                    nc.vector.reciprocal(recP[:, :], denP[:, :])
                    rec = tl.tile([1, S], RDT, tag="rec")
                    # f32r is byte-identical to f32: bitcast instead of a
                    # cast-DMA so this rides the fast HWDGE ring
                    nc.sync.dma_start(
                        out=rec[:, :].rearrange("p (a b) -> p a b", b=S // 128),
                        in_=recP[:, :].bitcast(RDT) if RDT != F32 else recP[:, :],
                    )
                    tails.append((stage, rec, hX, p, 0, S))

                # normalization (broadcast matmul + multiply) deferred by one
                # pair: the PE queue is strictly in-order, so the K=1
                # broadcast matmul must never wait on a fresh rec chain
                if len(tails) > 2:
                    emit_norm(tails[:-2])
                    tails = tails[-2:]

            def fc_mms(py, q, i_range, last):
                for i in i_range:
                    lhsT = outT_sb[:, S * i + 128 * q : S * i + 128 * (q + 1)]
                    for o0, ow in OSP:
                        nc.tensor.matmul(
                            py[:, o0 : o0 + ow],
                            lhsT,
                            fcw_sb[:, H * i + o0 : H * i + o0 + ow],
                            start=(i == 0), stop=(i == last),
                        )

            # the final pairs' normalization matmuls wait on their rec
            # chains; fill that in-order PE bubble with the first qtile's
            # early fc accumulation (itiles 0..ITILES-3 are long since
            # normalized and independent of the pending norms)
            py0 = ps.tile([128, H], F32, tag="s")
            fc_mms(py0, 0, range(ITILES - 2), ITILES - 1)
            emit_norm(tails)
            fc_mms(py0, 0, range(ITILES - 2, ITILES), ITILES - 1)
            ysb0 = yp.tile([128, H], F32, tag="ysb")
            nc.vector.tensor_add(ysb0[:, :], py0[:, :], fcb_sb[:, :])
            nc.sync.dma_start(out=y[0:128, :], in_=ysb0[:, :])

            for q in range(1, QTILES):
                py = ps.tile([128, H], F32, tag="s")
                fc_mms(py, q, range(ITILES), ITILES - 1)
                ysb = yp.tile([128, H], F32, tag="ysb")
                nc.vector.tensor_add(ysb[:, :], py[:, :], fcb_sb[:, :])
                nc.sync.dma_start(out=y[128 * q : 128 * (q + 1), :], in_=ysb[:, :])

    if split_waits:
        _split_excess_waits(nc)
    return nc


def prep_core_inputs(q_n, k_n, v_n, fc_wT, fc_b1, HEADS=16, mm_dtype="f32r"):
    """Host-side layout prep for one batch element."""
    import ml_dtypes

    cast = (lambda a: a.astype(ml_dtypes.bfloat16)) if mm_dtype == "bf16" else (lambda a: a)
    HD = 64
    S, H = q_n.shape
    KT = S // 128
    qT = np.ascontiguousarray(q_n.T)
    kT = np.ascontiguousarray(k_n.T)
    v4 = v_n.reshape(KT, 128, HEADS, HD)  # [t, p, h, c]
    vaug = np.empty((HEADS, 128, KT, HD + 1), dtype=np.float32)
    vaug[..., :HD] = v4.transpose(2, 1, 0, 3)
    vaug[..., HD] = 1.0
    return {
        "ones64": np.ones((1, 64), dtype=np.float32),
        "qT": cast(qT),
        "kT": cast(kT),
        "vaug": cast(np.ascontiguousarray(vaug.reshape(HEADS, 128, KT * (HD + 1)))),
        "fc_wT": cast(fc_wT),
        "fc_b": fc_b1,
    }


MM_DTYPE = "bf16"

_CACHED_NC = None


def _get_nc():
    global _CACHED_NC
    if _CACHED_NC is None:
        _CACHED_NC = build_kernel(mm_dtype=MM_DTYPE)
    return _CACHED_NC


def make_in_maps(key, value, query, fc_w, fc_b):
    key = np.asarray(key, dtype=np.float32)
    value = np.asarray(value, dtype=np.float32)
    query = np.asarray(query, dtype=np.float32)
    fc_w = np.asarray(fc_w, dtype=np.float32)
    fc_b = np.asarray(fc_b, dtype=np.float32)
    N, S, H = query.shape
    fc_wT = np.ascontiguousarray(fc_w.T)
    fc_b1 = np.ascontiguousarray(fc_b.reshape(1, H))
    return [
        prep_core_inputs(query[n], key[n], value[n], fc_wT, fc_b1, mm_dtype=MM_DTYPE)
        for n in range(N)
    ]


def run_on_device(in_maps):
    from concourse.bass_utils import run_bass_kernel_spmd

    nc = _get_nc()
    res = run_bass_kernel_spmd(nc, in_maps, list(range(N_CORES)))
    return np.stack([res.results[i]["y"] for i in range(N_CORES)], axis=0)


def kernel(key, value, query, fc_w, fc_b):
    """Full inputs in, full output out. Shards batch N=8 across 8 cores."""
    in_maps = make_in_maps(key, value, query, fc_w, fc_b)
    return run_on_device(in_maps)



# revision 30
# speedup vs baseline: 1.0577x; 1.0350x over previous
"""Trainium2 Bass kernel: 16-head attention (S=1024, hidden=1024) + output
linear, data-parallel over the batch dimension (8 batch elements -> 8 cores).

Contract: kernel(**inputs) takes the FULL unsharded inputs of
nn_Attention_83915071029891 and returns the FULL (8, 1024, 1024) f32 output.

Per-core algorithm (transposed-scores layout; TensorE contracts over the
partition dim, so scores are built k-major and never transposed):
  per head-pair (head A on partitions 0-63, head B on 64-127):
    scoresT[ktile] = kT-stationary.T @ qT        (row-packed A||B matmuls)
    attnT = exp(scoresT / sqrt(1024))            (ScalarE, reads PSUM)
    outTaug = [v | 1]-stationary.T @ attnT       (PSUM-accumulated over k)
      rows 0-63: unnormalized out^T; row 64: softmax denominator
    rows scaled by reciprocal(denominator)       (DVE + broadcast DMA)
  y[qtile] = outT-stationary.T @ fc_wT + fc_b    (PSUM-accumulated over i)
"""

import sys

for _p in ("/opt/trn_rl_repo", "/root/.axon_site/_ro/trn_rl_repo"):
    if _p not in sys.path:
        sys.path.append(_p)

from contextlib import ExitStack

import numpy as np

import bass_rust
import concourse.bass as bass
import concourse.mybir as mybir
import concourse.tile as tile
from concourse import library_config
from concourse.vector_clock import ScopedClock

F32 = mybir.dt.float32
AF = mybir.ActivationFunctionType

N_CORES = 8
_MAX_CTRL_WAITS = 1
ROWPACK_DEP = True


def _patched_drain_and_barrier(self, tick_clock, wait_clock):
    """Tile's kernel-tail Drain aggregates one sem wait per outstanding proc,
    but walrus CoreV3 codegen only has one sync-wait slot on CTRL ops -- split
    the waits across a chain of SP drain instructions."""
    nc = self.nc
    drain_inst = nc.sync.drain()
    wait_clock.add_sem_waits(
        drain_inst.ins, ScopedClock({None: tick_clock.global_clock})
    )
    si = drain_inst.ins.sync_info
    if si is not None and si.on_wait and len(si.on_wait) > _MAX_CTRL_WAITS:
        waits = list(si.on_wait)
        drain_inst.ins.sync_info = bass_rust.SyncInfo(
            on_wait=waits[:_MAX_CTRL_WAITS], on_update=list(si.on_update or [])
        )
        for i in range(_MAX_CTRL_WAITS, len(waits), _MAX_CTRL_WAITS):
            extra = nc.sync.drain()
            extra.ins.sync_info = bass_rust.SyncInfo(
                on_wait=waits[i : i + _MAX_CTRL_WAITS], on_update=[]
            )

    nc.all_engine_barrier()
    assert self.sems is not None
    popped = nc._tile_sem_poison_stack.pop()
    assert popped is self._sem_poison
    nc.clear_and_free_semaphores(list(self.sems.allocated().values()))
    nc.all_engine_barrier()


tile.TileContext._drain_and_barrier = _patched_drain_and_barrier


def _split_excess_waits(nc, max_waits=_MAX_CTRL_WAITS):
    """walrus CoreV3 setupSyncWait only has one sync-wait slot per
    instruction; hoist excess sem waits onto same-engine NoOp carriers
    inserted immediately before the over-limit instruction."""
    ctr = [0]

    def carrier(engine, waits):
        ctr[0] += 1
        nop = mybir.InstNoOp(name=f"I-waitc-{ctr[0]}", ins=[], outs=[])
        nop.engine = engine
        nop.sync_info = bass_rust.SyncInfo(on_wait=waits, on_update=[])
        return nop

    for fn in nc.m.functions:
        for blk in fn.blocks:
            il = blk.instructions
            newl = []
            changed = False
            for inst in il:
                si = inst.sync_info
                nw = len(si.on_wait) if si and si.on_wait else 0
                if nw > max_waits:
                    waits = list(si.on_wait)
                    for i in range(max_waits, len(waits), max_waits):
                        newl.append(carrier(inst.engine, waits[i : i + max_waits]))
                    inst.sync_info = bass_rust.SyncInfo(
                        on_wait=waits[:max_waits], on_update=list(si.on_update or [])
                    )
                    changed = True
                newl.append(inst)
            if changed:
                il.clear()
                il.extend(newl)
                assert len(blk.instructions) == len(newl), (
                    "block instruction list is not a live reference"
                )


def _splits(total, width):
    return [(s, min(width, total - s)) for s in range(0, total, width)]


def build_kernel(S=1024, HEADS=16, mm_dtype="f32r", split_waits=True):
    """Trace the per-core Bass program. DRAM io: qT,kT,vaug,fc_wT,fc_b -> y."""
    HD = 64
    H = HEADS * HD
    KT = S // 128
    PAIRS = HEADS // 2
    ITILES = H // 128
    QTILES = S // 128
    VW = HD + 1
    SCALE = 1.0 / float(H) ** 0.5
    NSP = _splits(S, 512)
    OSP = _splits(H, 512)

    nc = bass.Bass(trn_type="TRN2")

    # fp32 matmuls run as 2 half-speed passes (4 cyc/row). fp32r streams at
    # 1 cyc/row but does not register as PE activity for the HAM clock gate,
    # pinning the array at 1.2 GHz. bf16 runs the normal warm path.
    MMDT = {"f32r": mybir.dt.float32r, "bf16": mybir.dt.bfloat16, "f32": F32}[mm_dtype]
    # reciprocal-broadcast matmul operands: f32r keeps the full fp32 bytes
    # (bf16 would cost ~0.4% on every normalized output)
    RDT = F32 if mm_dtype == "f32" else mybir.dt.float32r

    qT = nc.dram_tensor("qT", [H, S], MMDT, kind="ExternalInput").ap()
    kT = nc.dram_tensor("kT", [H, S], MMDT, kind="ExternalInput").ap()
    vaug = nc.dram_tensor("vaug", [HEADS, 128, KT * VW], MMDT, kind="ExternalInput").ap()
    fc_wT = nc.dram_tensor("fc_wT", [H, H], MMDT, kind="ExternalInput").ap()
    fc_b = nc.dram_tensor("fc_b", [1, H], F32, kind="ExternalInput").ap()
    ones64 = nc.dram_tensor("ones64", [1, 64], RDT, kind="ExternalInput").ap()
    y = nc.dram_tensor("y", [S, H], F32, kind="ExternalOutput").ap()

    with tile.TileContext(nc) as tc:
        with ExitStack() as ctx:
            big = ctx.enter_context(tc.tile_pool(name="big", bufs=1))
            qk = ctx.enter_context(tc.tile_pool(name="qk", bufs=2))
            vp = ctx.enter_context(tc.tile_pool(name="vp", bufs=2))
            at = ctx.enter_context(tc.tile_pool(name="at", bufs=6))
            tl = ctx.enter_context(tc.tile_pool(name="tl", bufs=3))
            yp = ctx.enter_context(tc.tile_pool(name="yp", bufs=2))
            # PSUM budget (8 banks): "ps" 2 x (128,S)=4 banks, "po" oA+oB=4
            ps = ctx.enter_context(tc.tile_pool(name="ps", bufs=3, space="PSUM"))
            po = ctx.enter_context(tc.tile_pool(name="po", bufs=1, space="PSUM"))

            outT_sb = big.tile([128, ITILES * S], MMDT, tag="outT")
            ones1 = big.tile([1, 64], RDT, tag="ones1")
            nc.sync.dma_start(out=ones1[:, :], in_=ones64[:, :])
            fcw_sb = big.tile([128, ITILES * H], MMDT, tag="fcw")
            fcb_sb = big.tile([128, H], F32, tag="fcb")

            def load_pair(p, v_on_gpsimd=False):
                psl = slice(128 * p, 128 * (p + 1))
                qTp = qk.tile([128, S], MMDT, tag="qT")
                kTp = qk.tile([128, S], MMDT, tag="kT")
                if p == 0:
                    # small first chunks so the first QK matmul starts early
                    c0 = min(512, S)
                    nc.sync.dma_start(out=kTp[:, 0:128], in_=kT[psl, 0:128])
                    nc.sync.dma_start(out=qTp[:, 0:c0], in_=qT[psl, 0:c0])
                    nc.sync.dma_start(out=kTp[:, 128:S], in_=kT[psl, 128:S])
                    if c0 < S:
                        nc.sync.dma_start(out=qTp[:, c0:S], in_=qT[psl, c0:S])
                else:
                    nc.sync.dma_start(out=qTp[:, :], in_=qT[psl, :])
                    nc.sync.dma_start(out=kTp[:, :], in_=kT[psl, :])
                # the first pairs' V loads ride the gpsimd ring (ahead of
                # the fc weights) so the cold-start DMA deliveries come off
                # two rings in parallel -- the lead-in is paced by serial
                # per-DMA latency, not bandwidth
                eng = nc.gpsimd if v_on_gpsimd else nc.sync
                vA = vp.tile([128, KT * VW], MMDT, tag="vA")
                eng.dma_start(out=vA[:, :], in_=vaug[2 * p])
                vB = vp.tile([128, KT * VW], MMDT, tag="vB")
                eng.dma_start(out=vB[:, :], in_=vaug[2 * p + 1])
                return qTp, kTp, vA, vB

            nxt = load_pair(0, v_on_gpsimd=True)
            nxt_pre = load_pair(1, v_on_gpsimd=True)
            # fc weights follow the first pairs' V loads on the GPSIMD SWDGE
            # ring; they are not needed until the FC phase ~160us in.
            for i in range(ITILES):
                nc.gpsimd.dma_start(
                    out=fcw_sb[:, H * i : H * (i + 1)],
                    in_=fc_wT[128 * i : 128 * (i + 1), :],
                )
            nc.gpsimd.dma_start(
                out=fcb_sb[:, :], in_=fc_b.unsqueeze(1).broadcast_to((1, 128, H))
            )
            tails = []

            def emit_norm(entries):
                for stage, rec, hX, p_, n0, nw in entries:
                    # R = ones(64)^T @ rec : per-q reciprocal broadcast to 64
                    # partitions via a K=1 matmul into a spare PSUM slot
                    Rp = ps.tile([64, nw], F32, tag="s")
                    for m0, mw in _splits(nw, 512):
                        nc.tensor.matmul(
                            Rp[:, m0 : m0 + mw], ones1[:, :],
                            rec[0:1, m0 : m0 + mw], start=True, stop=True,
                        )
                    osl = slice(S * p_ + n0, S * p_ + n0 + nw)
                    nsl_ = slice(n0, n0 + nw)
                    if hX % 2 == 0:
                        nc.vector.tensor_mul(
                            outT_sb[0:64, osl], stage[0:64, nsl_], Rp[:, :]
                        )
                    else:
                        stg = tl.tile([64, nw], MMDT, tag="stg")
                        nc.vector.tensor_mul(stg[:, :], stage[0:64, nsl_], Rp[:, :])
                        nc.sync.dma_start(out=outT_sb[64:128, osl], in_=stg[:, :])

            for p in range(PAIRS):
                hA, hB = 2 * p, 2 * p + 1
                qTp, kTp, vA, vB = nxt
                if p == 0:
                    nxt = nxt_pre
                elif p + 1 < PAIRS:
                    nxt = load_pair(p + 1)
                stageA = tl.tile([65, S], F32, tag="stageA")
                stageB = tl.tile([65, S], F32, tag="stageB")
                # q-dim processed in 512-wide passes: each (ktile, pass)
                # emits one [scoresT_A | scoresT_B] PSUM tile so a single
                # FD=1024 ACT exp covers both heads, and the AV accumulators
                # only need one PSUM bank per head. Each pass is normalized
                # as soon as its AV accumulation finishes.
                for n0, nw in NSP:
                    nsl = slice(n0, n0 + nw)
                    po_A = po.tile([VW, nw], F32, tag="oA")
                    po_B = po.tile([VW, nw], F32, tag="oB")
                    for t in range(KT):
                        ksl = slice(128 * t, 128 * (t + 1))
                        sAB = ps.tile([128, 2 * nw], F32, tag="s")
                        mmA = nc.tensor.matmul(
                            sAB[:, 0:nw], kTp[0:64, ksl], qTp[0:64, nsl],
                            start=True, stop=True,
                        )
                        mmB = nc.tensor.matmul(
                            sAB[:, nw : 2 * nw], kTp[64:128, ksl], qTp[64:128, nsl],
                            start=True, stop=True,
                        )
                        # keep the K=64 row-group pair adjacent on PE so the
                        # two half-array matmuls run concurrently
                        if ROWPACK_DEP:
                            tile.add_dep_helper(
                                mmB.ins, mmA.ins, sync=False, reason="rowpack"
                            )
                        aAB = at.tile([128, 2 * nw], MMDT, tag="a")
                        nc.scalar.activation(aAB[:, :], sAB[:, :], AF.Exp, scale=SCALE)
                        vsl = slice(VW * t, VW * (t + 1))
                        nc.tensor.matmul(
                            po_A[:, :], vA[:, vsl], aAB[:, 0:nw],
                            start=(t == 0), stop=(t == KT - 1),
                        )
                        nc.tensor.matmul(
                            po_B[:, :], vB[:, vsl], aAB[:, nw : 2 * nw],
                            start=(t == 0), stop=(t == KT - 1),
                        )
                    for po_X, stage in ((po_A, stageA), (po_B, stageB)):
                        # evacuate the accumulator so the PSUM bank frees for
                        # the next q-pass; normalization happens per pair
                        nc.vector.tensor_copy(stage[:, nsl], po_X[:, :])

                for stage, hX in ((stageA, hA), (stageB, hB)):
                    # 1/denominator: reshape the row to (128, S/128) so the
                    # iterative RECIPROCAL walks few elements per lane
                    denP = tl.tile([128, S // 128], F32, tag="denP")
                    nc.sync.dma_start(
                        out=denP[:, :],
                        in_=stage[64:65, :].rearrange("p (a b) -> p a b", b=S // 128),
                    )
                    recP = tl.tile([128, S // 128], F32, tag="recP")
                    nc.vector.reciprocal(recP[:, :], denP[:, :])
                    rec = tl.tile([1, S], RDT, tag="rec")
                    # f32r is byte-identical to f32: bitcast instead of a
                    # cast-DMA so this rides the fast HWDGE ring
                    nc.sync.dma_start(
                        out=rec[:, :].rearrange("p (a b) -> p a b", b=S // 128),
                        in_=recP[:, :].bitcast(RDT) if RDT != F32 else recP[:, :],
                    )
                    tails.append((stage, rec, hX, p, 0, S))

                # normalization (broadcast matmul + multiply) deferred by one
                # pair: the PE queue is strictly in-order, so the K=1
                # broadcast matmul must never wait on a fresh rec chain
                if len(tails) > 2:
                    emit_norm(tails[:-2])
                    tails = tails[-2:]

            def fc_mms(py, q, i_range, last):
                for i in i_range:
                    lhsT = outT_sb[:, S * i + 128 * q : S * i + 128 * (q + 1)]
                    for o0, ow in OSP:
                        nc.tensor.matmul(
                            py[:, o0 : o0 + ow],
                            lhsT,
                            fcw_sb[:, H * i + o0 : H * i + o0 + ow],
                            start=(i == 0), stop=(i == last),
                        )

            # the final pairs' normalization matmuls wait on their rec
            # chains; fill that in-order PE bubble with the first qtile's
            # early fc accumulation (itiles 0..ITILES-3 are long since
            # normalized and independent of the pending norms)
            py0 = ps.tile([128, H], F32, tag="s")
            fc_mms(py0, 0, range(ITILES - 2), ITILES - 1)
            emit_norm(tails)
            fc_mms(py0, 0, range(ITILES - 2, ITILES), ITILES - 1)
            ysb0 = yp.tile([128, H], F32, tag="ysb")
            nc.vector.tensor_add(ysb0[:, :], py0[:, :], fcb_sb[:, :])
            nc.sync.dma_start(out=y[0:128, :], in_=ysb0[:, :])

            for q in range(1, QTILES):
                py = ps.tile([128, H], F32, tag="s")
                fc_mms(py, q, range(ITILES), ITILES - 1)
                ysb = yp.tile([128, H], F32, tag="ysb")
                nc.vector.tensor_add(ysb[:, :], py[:, :], fcb_sb[:, :])
                nc.sync.dma_start(out=y[128 * q : 128 * (q + 1), :], in_=ysb[:, :])

    if split_waits:
        _split_excess_waits(nc)
    return nc


def prep_core_inputs(q_n, k_n, v_n, fc_wT, fc_b1, HEADS=16, mm_dtype="f32r"):
    """Host-side layout prep for one batch element."""
    import ml_dtypes

    cast = (lambda a: a.astype(ml_dtypes.bfloat16)) if mm_dtype == "bf16" else (lambda a: a)
    HD = 64
    S, H = q_n.shape
    KT = S // 128
    qT = np.ascontiguousarray(q_n.T)
    kT = np.ascontiguousarray(k_n.T)
    v4 = v_n.reshape(KT, 128, HEADS, HD)  # [t, p, h, c]
    vaug = np.empty((HEADS, 128, KT, HD + 1), dtype=np.float32)
    vaug[..., :HD] = v4.transpose(2, 1, 0, 3)
    vaug[..., HD] = 1.0
    return {
        "ones64": np.ones((1, 64), dtype=np.float32),
        "qT": cast(qT),
        "kT": cast(kT),
        "vaug": cast(np.ascontiguousarray(vaug.reshape(HEADS, 128, KT * (HD + 1)))),
        "fc_wT": cast(fc_wT),
        "fc_b": fc_b1,
    }


MM_DTYPE = "bf16"

_CACHED_NC = None


def _get_nc():
    global _CACHED_NC
    if _CACHED_NC is None:
        _CACHED_NC = build_kernel(mm_dtype=MM_DTYPE)
    return _CACHED_NC


def make_in_maps(key, value, query, fc_w, fc_b):
    key = np.asarray(key, dtype=np.float32)
    value = np.asarray(value, dtype=np.float32)
    query = np.asarray(query, dtype=np.float32)
    fc_w = np.asarray(fc_w, dtype=np.float32)
    fc_b = np.asarray(fc_b, dtype=np.float32)
    N, S, H = query.shape
    fc_wT = np.ascontiguousarray(fc_w.T)
    fc_b1 = np.ascontiguousarray(fc_b.reshape(1, H))
    return [
        prep_core_inputs(query[n], key[n], value[n], fc_wT, fc_b1, mm_dtype=MM_DTYPE)
        for n in range(N)
    ]


def run_on_device(in_maps):
    from concourse.bass_utils import run_bass_kernel_spmd

    nc = _get_nc()
    res = run_bass_kernel_spmd(nc, in_maps, list(range(N_CORES)))
    return np.stack([res.results[i]["y"] for i in range(N_CORES)], axis=0)


def kernel(key, value, query, fc_w, fc_b):
    """Full inputs in, full output out. Shards batch N=8 across 8 cores."""
    in_maps = make_in_maps(key, value, query, fc_w, fc_b)
    return run_on_device(in_maps)

